# revision 25
# baseline (speedup 1.0000x reference)
"""Trainium2 Bass kernel for PoseOptimizerLayer's build_q_matrix.

Math: every entry of the (5,5) Q is a bilinear form in per-point features
  phi(a_i) = [1, x_a, y_a, x_a^2+y_a^2]   (Na x 4)
  psi(b_j) = [1, x_b, y_b, x_b^2+y_b^2]   (Nb x 4)
through the association-weighted moment matrix
  S = phi^T A psi                          (4 x 4 per batch)
and Q_flat(25) = TmatQ^T @ s_flat for a constant TmatQ.

Device plan (per core, 2 of the 16 batches; data-parallel over batch, no
collectives).  The kernel is HBM-bound (32MB of associations per core,
~358 GB/s/NC limit), so the point of the design is to keep the PE off the
critical path and stream A at full rate:

  stage 1: P32 = PhiHL^T A   (32 x Nb) -- PE matmuls in float32r (1 cycle/row
           at moving width 512, i.e. 4x the fp32 rate).  The (128 x 32)
           stationary tile holds phi split into f32r hi+lo halves, each
           replicated 4x (col 16h+4q+pp = phi_pp part h): the split makes
           the phi side of the product exact (hi+lo == phi in fp32), and the
           replication makes the PSUM output land directly in the layout
           stage 2 wants.  A streams on the sync HWDGE queue alone (343
           GB/s/core measured; alternating sync+scalar drops to 288) and is
           rounded f32->f32r by copies alternating between the vector and
           scalar engines (the BIR verifier requires f32r matmul operands to
           be produced pre-rounded).  Accumulation over the 16 i-chunks in 4
           one-bank PSUM tiles (32 x 512).
  stage 2: fused DVE tensor_tensor_reduce per PSUM tile against psi rows
           replicated on 32 partitions (g_rep, built early via scalar-queue
           DMAs): s32[:, jc] = sum_j P32[:, j] g_rep[:, j].
  stage 3: q_part(25, 4) = TmatQ2^T @ s32 -- the (32, 25) stationary
           [TmatQ; TmatQ] folds the hi+lo halves; a final 4-wide DVE reduce
           sums the per-bank partials into Q_flat(25).
"""

import os
import numpy as np

BATCH, NA, NB = 16, 2048, 2048
N_CORES = 8
BL = BATCH // N_CORES  # batches per core
P = 128
IC = NA // P  # i-chunks
NJ = 512      # moving-operand width (fp32 max, = one PSUM bank)
JC = NB // NJ  # j-chunks of the stage-1 moving operand

A_BUFS = int(os.environ.get("KERNEL_A_BUFS", "5"))
DMA_CH = int(os.environ.get("KERNEL_DMA_CH", "2"))  # i-chunks per A DMA
DEBUG_STAGE = int(os.environ.get("KERNEL_DEBUG_STAGE", "0"))
USE_TTR = os.environ.get("KERNEL_TTR", "0") == "1"

LAST_RESULTS = None  # test harness can inspect exec_time_ns etc.


def _tmatq() -> np.ndarray:
    """(16, 25): row 4q+pp = coeff of S[pp][q] in Q_flat[k]."""
    T = np.zeros((16, 25), np.float32)

    def s(p, q):
        return 4 * p + q

    entries = [
        (s(0, 3), 0, 1.0),                      # q00 = S03
        (s(0, 1), 1, -1.0), (s(0, 1), 5, -1.0),   # q01 = -S01
        (s(0, 2), 2, -1.0), (s(0, 2), 10, -1.0),  # q02 = -S02
        (s(1, 1), 3, -1.0), (s(2, 2), 3, -1.0),   # q03 = -(S11+S22)
        (s(1, 1), 15, -1.0), (s(2, 2), 15, -1.0),
        (s(2, 1), 4, 1.0), (s(1, 2), 4, -1.0),    # q04 = S21-S12
        (s(2, 1), 20, 1.0), (s(1, 2), 20, -1.0),
        (s(0, 0), 6, 1.0), (s(0, 0), 12, 1.0),    # w = S00
        (s(1, 0), 8, 1.0), (s(1, 0), 16, 1.0),    # q13 = q24 = S10
        (s(1, 0), 14, 1.0), (s(1, 0), 22, 1.0),
        (s(2, 0), 9, -1.0), (s(2, 0), 21, -1.0),  # q14 = -S20
        (s(2, 0), 13, 1.0), (s(2, 0), 17, 1.0),   # q23 = S20
        (s(3, 0), 18, 1.0), (s(3, 0), 24, 1.0),   # q33 = S30
    ]
    for si, qi, v in entries:
        T[si, qi] += v
    TQ = np.zeros((16, 25), np.float32)
    for pp in range(4):
        for q in range(4):
            TQ[4 * q + pp] = T[4 * pp + q]
    return TQ


_BUILT = None


def _build():
    global _BUILT
    if _BUILT is not None:
        return _BUILT
    import concourse.bass as bass
    import concourse.mybir as mybir
    import concourse.tile as tile
    from concourse import bacc

    f32 = mybir.dt.float32
    f32r = mybir.dt.float32r
    alu = mybir.AluOpType

    nc = bacc.Bacc("TRN2", target_bir_lowering=False, debug=False)
    # Declared f32r: same bit layout as the f32 input; the PE's f32r
    # datapath uses the high 20 bits (truncation instead of
    # round-to-nearest -- fine at the 2e-2 gate, measured 1.5e-3).
    A = nc.dram_tensor("associations", [BL, NA, NB], f32r, kind="ExternalInput")
    pa = nc.dram_tensor("pt_in_a", [BL, NA, 2], f32, kind="ExternalInput")
    pb = nc.dram_tensor("pt_in_b", [BL, NB, 2], f32, kind="ExternalInput")
    tm = nc.dram_tensor("tmatq", [32, 25], f32, kind="ExternalInput")
    qo = nc.dram_tensor("q_out", [BL, 5, 5], f32, kind="ExternalOutput")

    with tile.TileContext(nc) as tc:
        with (
            tc.tile_pool(name="const", bufs=1) as cpool,
            tc.tile_pool(name="feat", bufs=2) as fpool,
            tc.tile_pool(name="scratch", bufs=1) as s1pool,
            tc.tile_pool(name="abuf", bufs=A_BUFS) as apool,
            tc.tile_pool(name="small", bufs=2) as spool,
            tc.tile_pool(name="psp", bufs=1, space=bass.MemorySpace.PSUM) as psp,
            tc.tile_pool(name="pss", bufs=2, space=bass.MemorySpace.PSUM) as pss,
        ):
            tmat_sb = cpool.tile([32, 25], f32, tag="tmat")
            nc.scalar.dma_start(tmat_sb[:], tm[:])

            s_tiles = []
            feats = []
            # ---- prologue: features for BOTH batches, so batch 1's vector
            # ops are not queued behind batch 0's stage 2 in the DVE FIFO
            # (that ordering cost ~12us of PE idle at the batch boundary)
            for b in range(BL):
                # phi features, planar planes [1 | x | y | x^2+y^2] of
                # width IC.  The i-order is permuted: chunk c covers rows
                # i = p*16 + c (i.e. A rows c::16), so pt_in_a loads as ONE
                # contiguous (128, 32) DMA -- the naive (c p)->p layout
                # generates 4096 4-byte descriptors that clog all 16 SDMA
                # engines for ~15us and starve the A stream.
                pa_c = fpool.tile([P, 2 * IC], f32, tag="pac")
                nc.scalar.dma_start(
                    pa_c[:], pa[b].rearrange("(p c) k -> p (c k)", p=P)
                )
                pav = pa_c[:].rearrange("p (c k) -> p k c", k=2)
                f_st = fpool.tile([P, 4 * IC], f32, tag="fstg")
                nc.vector.memset(f_st[:, 0:IC], 1.0)
                nc.vector.tensor_copy(f_st[:, IC : 2 * IC], pav[:, 0, :])
                nc.vector.tensor_copy(f_st[:, 2 * IC : 3 * IC], pav[:, 1, :])
                ftmp = fpool.tile([P, IC], f32, tag="ftmp")
                nc.vector.tensor_mul(f_st[:, 3 * IC : 4 * IC], f_st[:, IC : 2 * IC],
                                     f_st[:, IC : 2 * IC])
                nc.vector.tensor_mul(ftmp[:], f_st[:, 2 * IC : 3 * IC],
                                     f_st[:, 2 * IC : 3 * IC])
                nc.vector.tensor_add(f_st[:, 3 * IC : 4 * IC],
                                     f_st[:, 3 * IC : 4 * IC], ftmp[:])
                # split phi = hi + lo (both exactly representable in f32r):
                # hi = round11(phi), lo = phi - hi (the residual has <= 12
                # significant bits, so its f32r rounding is exact)
                f_hi = fpool.tile([P, 4 * IC], f32r, tag="fhi")
                nc.vector.tensor_copy(f_hi[:], f_st[:])
                f_lo = fpool.tile([P, 4 * IC], f32, tag="flo")
                nc.vector.tensor_sub(f_lo[:], f_st[:], f_hi[:].bitcast(f32))
                # interleave to (c, h, q, f): stationary chunk ic is the
                # contiguous (128, 32) slice with col 16h+4q+pp = phi_pp
                # part h -- the matmul then emits P replicated over q and
                # split over h for free.
                f_sb = fpool.tile([P, 32 * IC], f32r, tag="f")
                fview = f_sb[:].rearrange("p (c h q f) -> p h q c f", h=2, q=4, f=4)
                for h, src in ((0, f_hi[:].bitcast(f32)), (1, f_lo[:])):
                    srcv = src.rearrange("p (f c) -> p c f", c=IC)
                    for q in range(4):
                        nc.vector.tensor_copy(fview[:, h, q], srcv)

                # ---- psi rows: [x | y | x^2+y^2] built on one partition,
                # then scattered to (32, NB), row 16h+4q+pp = psi_q
                pb_row = s1pool.tile([1, 2 * NB], f32, tag="pbrow")
                nc.scalar.dma_start(
                    pb_row[:], pb[b].rearrange("j k -> (j k)")[None, :]
                )
                grow = s1pool.tile([1, 3 * NB], f32, tag="grow")
                prview = pb_row[:].rearrange("p (j k) -> p k j", k=2)
                nc.vector.tensor_copy(grow[:, 0:NB], prview[:, 0, :])
                nc.vector.tensor_copy(grow[:, NB : 2 * NB], prview[:, 1, :])
                nc.vector.tensor_mul(grow[:, 2 * NB : 3 * NB], grow[:, 0:NB],
                                     grow[:, 0:NB])
                gtmp = s1pool.tile([1, NB], f32, tag="gtmp")
                nc.vector.tensor_mul(gtmp[:], grow[:, NB : 2 * NB],
                                     grow[:, NB : 2 * NB])
                nc.vector.tensor_add(grow[:, 2 * NB : 3 * NB],
                                     grow[:, 2 * NB : 3 * NB], gtmp[:])
                g_rep = fpool.tile([32, NB], f32, tag="grep")
                ones_row = s1pool.tile([1, NB], f32, tag="ones")
                nc.vector.memset(ones_row[:], 1.0)
                nc.vector.memset(g_rep[0:4, :], 1.0)
                for h in range(2):
                    for q in range(4):
                        for pp in range(4):
                            r = 16 * h + 4 * q + pp
                            if r < 4:
                                continue  # covered by the memset
                            src = (
                                ones_row[:, :]
                                if q == 0
                                else grow[:, (q - 1) * NB : q * NB]
                            )
                            nc.scalar.dma_start(g_rep[r : r + 1, :], src)
                feats.append((f_sb, g_rep))

            for b in range(BL):
                f_sb, g_rep = feats[b]
                # ---- stage 1: P32(32, NB) accumulated in 4 one-bank PSUM
                # tiles.  A streams on the sync queue in 2-chunk (2MB) DMAs:
                # in the permuted i-order, adjacent chunks are adjacent DRAM
                # rows, so each partition reads 16KB contiguous.
                p_banks = [
                    psp.tile([32, NJ], f32, tag=f"p{jc}", name=f"p{jc}")
                    for jc in range(JC)
                ]
                Aview = A[b].rearrange("(p s) j -> p s j", s=IC)
                for ic0 in range(0, IC, DMA_CH):
                    a_t = apool.tile([P, DMA_CH * NB], f32r, tag="a")
                    nc.sync.dma_start(
                        a_t[:].rearrange("p (s j) -> p s j", j=NB),
                        Aview[:, ic0 : ic0 + DMA_CH, :],
                    )
                    for s in range(DMA_CH):
                        ic = ic0 + s
                        lhs = f_sb[:, ic * 32 : (ic + 1) * 32]
                        for jc in range(JC):
                            nc.tensor.matmul(
                                p_banks[jc][:],
                                lhs,
                                a_t[:, s * NB + jc * NJ : s * NB + (jc + 1) * NJ],
                                start=(ic == 0),
                                stop=(ic == IC - 1),
                            )

                if DEBUG_STAGE == 1:
                    # stop after stage 1: dump first 25 cols of P32 row 0
                    dbg = spool.tile([1, 25], f32, tag="dbg")
                    nc.vector.tensor_copy(dbg[:], p_banks[0][0:1, 0:25])
                    nc.scalar.dma_start(
                        qo[b].rearrange("a b -> () (a b)"), dbg[:]
                    )
                    continue

                # ---- stage 2: per-bank multiply against g_rep off PSUM,
                # then reduce -> s32
                w32 = spool.tile([32, NB], f32, tag="w32")
                if USE_TTR:
                    s_sb = spool.tile([32, JC], f32, tag="ssb")
                    for jc in range(JC):
                        nc.vector.tensor_tensor_reduce(
                            w32[:, jc * NJ : (jc + 1) * NJ],
                            p_banks[jc][:],
                            g_rep[:, jc * NJ : (jc + 1) * NJ],
                            1.0,
                            0.0,
                            alu.mult,
                            alu.add,
                            s_sb[:, jc : jc + 1],
                        )
                else:
                    s_sb = spool.tile([32, 1], f32, tag="ssb")
                    for jc in range(JC):
                        nc.vector.tensor_mul(
                            w32[:, jc * NJ : (jc + 1) * NJ],
                            p_banks[jc][:],
                            g_rep[:, jc * NJ : (jc + 1) * NJ],
                        )
                    nc.vector.tensor_reduce(
                        s_sb[:], w32[:], mybir.AxisListType.X, alu.add
                    )
                s_tiles.append(s_sb)

            # ---- stage 3 (epilogue, off the PE stream so batch 1's matmuls
            # are not queued behind it): q = tmatq2^T @ s32 (the duplicated
            # tmat rows fold the hi+lo halves over the contraction)
            for b, s_sb in enumerate(s_tiles):
                q_ps = pss.tile([25, s_sb.shape[1]], f32, tag="q")
                nc.tensor.matmul(q_ps[:], tmat_sb[:], s_sb[:], start=True, stop=True)
                q_sb = spool.tile([25, 1], f32, tag="qsb")
                if USE_TTR:
                    nc.vector.tensor_reduce(
                        q_sb[:], q_ps[:], mybir.AxisListType.X, alu.add
                    )
                else:
                    nc.vector.tensor_copy(q_sb[:], q_ps[:])
                nc.gpsimd.dma_start(qo[b].rearrange("a b -> (a b)"), q_sb[:, 0])

    nc.compile()
    _BUILT = nc
    return nc


def kernel(associations: np.ndarray, pt_in_a: np.ndarray, pt_in_b: np.ndarray
           ) -> np.ndarray:
    global LAST_RESULTS
    from concourse.bass_utils import run_bass_kernel_spmd

    nc = _build()
    tq = _tmatq()
    tmatq = np.concatenate([tq, tq], axis=0)  # (32, 25): folds hi+lo halves
    associations = np.ascontiguousarray(associations, dtype=np.float32)
    pt_in_a = np.ascontiguousarray(pt_in_a, dtype=np.float32)
    pt_in_b = np.ascontiguousarray(pt_in_b, dtype=np.float32)

    in_maps = []
    for c in range(N_CORES):
        sl = slice(c * BL, (c + 1) * BL)
        in_maps.append(
            {
                "associations": associations[sl],
                "pt_in_a": pt_in_a[sl],
                "pt_in_b": pt_in_b[sl],
                "tmatq": tmatq,
            }
        )
    res = run_bass_kernel_spmd(nc, in_maps, list(range(N_CORES)))
    LAST_RESULTS = res
    out = np.concatenate([res.results[c]["q_out"] for c in range(N_CORES)], axis=0)
    return out.astype(np.float32, copy=False)


# revision 29
# speedup vs baseline: 1.0885x; 1.0885x over previous
"""Trainium2 Bass kernel for PoseOptimizerLayer's build_q_matrix.

Math: every entry of the (5,5) Q is a bilinear form in per-point features
  phi(a_i) = [1, x_a, y_a, x_a^2+y_a^2]   (Na x 4)
  psi(b_j) = [1, x_b, y_b, x_b^2+y_b^2]   (Nb x 4)
through the association-weighted moment matrix
  S = phi^T A psi                          (4 x 4 per batch)
and Q_flat(25) = TmatQ^T @ s_flat for a constant TmatQ.

Device plan (per core, 2 of the 16 batches; data-parallel over batch, no
collectives).  The kernel is HBM-bound (32MB of associations per core,
~358 GB/s/NC limit), so the point of the design is to keep the PE off the
critical path and stream A at full rate:

  stage 1: P32 = PhiHL^T A   (32 x Nb) -- PE matmuls in float32r (1 cycle/row
           at moving width 512, i.e. 4x the fp32 rate).  The (128 x 32)
           stationary tile holds phi split into f32r hi+lo halves, each
           replicated 4x (col 16h+4q+pp = phi_pp part h): the split makes
           the phi side of the product exact (hi+lo == phi in fp32), and the
           replication makes the PSUM output land directly in the layout
           stage 2 wants.  A streams on the sync HWDGE queue alone (343
           GB/s/core measured; alternating sync+scalar drops to 288) and is
           rounded f32->f32r by copies alternating between the vector and
           scalar engines (the BIR verifier requires f32r matmul operands to
           be produced pre-rounded).  Accumulation over the 16 i-chunks in 4
           one-bank PSUM tiles (32 x 512).
  stage 2: fused DVE tensor_tensor_reduce per PSUM tile against psi rows
           replicated on 32 partitions (g_rep, built early via scalar-queue
           DMAs): s32[:, jc] = sum_j P32[:, j] g_rep[:, j].
  stage 3: q_part(25, 4) = TmatQ2^T @ s32 -- the (32, 25) stationary
           [TmatQ; TmatQ] folds the hi+lo halves; a final 4-wide DVE reduce
           sums the per-bank partials into Q_flat(25).
"""

import os
import numpy as np

BATCH, NA, NB = 16, 2048, 2048
N_CORES = 8
BL = BATCH // N_CORES  # batches per core
P = 128
IC = NA // P  # i-chunks
NJ = 512      # moving-operand width (fp32 max, = one PSUM bank)
JC = NB // NJ  # j-chunks of the stage-1 moving operand

A_BUFS = int(os.environ.get("KERNEL_A_BUFS", "5"))
DMA_CH = int(os.environ.get("KERNEL_DMA_CH", "2"))  # i-chunks per A DMA
DEBUG_STAGE = int(os.environ.get("KERNEL_DEBUG_STAGE", "0"))
USE_TTR = os.environ.get("KERNEL_TTR", "0") == "1"

LAST_RESULTS = None  # test harness can inspect exec_time_ns etc.


def _tmatq() -> np.ndarray:
    """(16, 25): row 4pp+q = coeff of S[pp][q] in Q_flat[k]."""
    T = np.zeros((16, 25), np.float32)

    def s(p, q):
        return 4 * p + q

    entries = [
        (s(0, 3), 0, 1.0),                      # q00 = S03
        (s(0, 1), 1, -1.0), (s(0, 1), 5, -1.0),   # q01 = -S01
        (s(0, 2), 2, -1.0), (s(0, 2), 10, -1.0),  # q02 = -S02
        (s(1, 1), 3, -1.0), (s(2, 2), 3, -1.0),   # q03 = -(S11+S22)
        (s(1, 1), 15, -1.0), (s(2, 2), 15, -1.0),
        (s(2, 1), 4, 1.0), (s(1, 2), 4, -1.0),    # q04 = S21-S12
        (s(2, 1), 20, 1.0), (s(1, 2), 20, -1.0),
        (s(0, 0), 6, 1.0), (s(0, 0), 12, 1.0),    # w = S00
        (s(1, 0), 8, 1.0), (s(1, 0), 16, 1.0),    # q13 = q24 = S10
        (s(1, 0), 14, 1.0), (s(1, 0), 22, 1.0),
        (s(2, 0), 9, -1.0), (s(2, 0), 21, -1.0),  # q14 = -S20
        (s(2, 0), 13, 1.0), (s(2, 0), 17, 1.0),   # q23 = S20
        (s(3, 0), 18, 1.0), (s(3, 0), 24, 1.0),   # q33 = S30
    ]
    for si, qi, v in entries:
        T[si, qi] += v
    return T


_BUILT = None


def _build():
    global _BUILT
    if _BUILT is not None:
        return _BUILT
    import concourse.bass as bass
    import concourse.mybir as mybir
    import concourse.tile as tile
    from concourse import bacc

    f32 = mybir.dt.float32
    f32r = mybir.dt.float32r
    alu = mybir.AluOpType

    nc = bacc.Bacc("TRN2", target_bir_lowering=False, debug=False)
    # Declared f32r: same bit layout as the f32 input; the PE's f32r
    # datapath uses the high 20 bits (truncation instead of
    # round-to-nearest -- fine at the 2e-2 gate, measured 1.5e-3).
    A = nc.dram_tensor("associations", [BL, NA, NB], f32r, kind="ExternalInput")
    pa = nc.dram_tensor("pt_in_a", [BL, NA, 2], f32, kind="ExternalInput")
    pb = nc.dram_tensor("pt_in_b", [BL, NB, 2], f32, kind="ExternalInput")
    tm = nc.dram_tensor("tmatq", [32, 25], f32, kind="ExternalInput")
    qo = nc.dram_tensor("q_out", [BL, 5, 5], f32, kind="ExternalOutput")

    with tile.TileContext(nc) as tc:
        with (
            tc.tile_pool(name="const", bufs=1) as cpool,
            tc.tile_pool(name="feat", bufs=2) as fpool,
            tc.tile_pool(name="scratch", bufs=1) as s1pool,
            tc.tile_pool(name="abuf", bufs=A_BUFS) as apool,
            tc.tile_pool(name="small", bufs=2) as spool,
            tc.tile_pool(name="psp", bufs=1, space=bass.MemorySpace.PSUM) as psp,
            tc.tile_pool(name="pss", bufs=2, space=bass.MemorySpace.PSUM) as pss,
        ):
            tmat_sb = cpool.tile([32, 25], f32, tag="tmat")
            nc.scalar.dma_start(tmat_sb[:], tm[:])

            s_tiles = []
            feats = []
            # ---- prologue: features for BOTH batches, so batch 1's vector
            # ops are not queued behind batch 0's stage 2 in the DVE FIFO
            # (that ordering cost ~12us of PE idle at the batch boundary)
            for b in range(BL):
                # phi features, planar planes [1 | x | y | x^2+y^2] of
                # width IC.  The i-order is permuted: chunk c covers rows
                # i = p*16 + c (i.e. A rows c::16), so pt_in_a loads as ONE
                # contiguous (128, 32) DMA -- the naive (c p)->p layout
                # generates 4096 4-byte descriptors that clog all 16 SDMA
                # engines for ~15us and starve the A stream.
                pa_c = fpool.tile([P, 2 * IC], f32, tag="pac")
                nc.scalar.dma_start(
                    pa_c[:], pa[b].rearrange("(p c) k -> p (c k)", p=P)
                )
                pav = pa_c[:].rearrange("p (c k) -> p k c", k=2)
                f_st = fpool.tile([P, 4 * IC], f32, tag="fstg")
                nc.vector.memset(f_st[:, 0:IC], 1.0)
                nc.vector.tensor_copy(f_st[:, IC : 2 * IC], pav[:, 0, :])
                nc.vector.tensor_copy(f_st[:, 2 * IC : 3 * IC], pav[:, 1, :])
                ftmp = fpool.tile([P, IC], f32, tag="ftmp")
                nc.vector.tensor_mul(f_st[:, 3 * IC : 4 * IC], f_st[:, IC : 2 * IC],
                                     f_st[:, IC : 2 * IC])
                nc.vector.tensor_mul(ftmp[:], f_st[:, 2 * IC : 3 * IC],
                                     f_st[:, 2 * IC : 3 * IC])
                nc.vector.tensor_add(f_st[:, 3 * IC : 4 * IC],
                                     f_st[:, 3 * IC : 4 * IC], ftmp[:])
                # split phi = hi + lo (both exactly representable in f32r):
                # hi = round11(phi), lo = phi - hi (the residual has <= 12
                # significant bits, so its f32r rounding is exact)
                f_hi = fpool.tile([P, 4 * IC], f32r, tag="fhi")
                nc.vector.tensor_copy(f_hi[:], f_st[:])
                f_lo = fpool.tile([P, 4 * IC], f32, tag="flo")
                nc.vector.tensor_sub(f_lo[:], f_st[:], f_hi[:].bitcast(f32))
                # interleave to (c, h, pp, q): stationary chunk ic is the
                # contiguous (128, 32) slice with col 16h+4pp+q = phi_pp
                # part h -- the matmul then emits P replicated over q and
                # split over h for free.  q-minor ordering makes each g_rep
                # 4-row group equal [psi_0..psi_3] = one contiguous 4-part
                # DMA from the staging row, no per-row scatter.
                f_sb = fpool.tile([P, 32 * IC], f32r, tag="f")
                fview = f_sb[:].rearrange(
                    "p (c h pp q) -> p h q c pp", h=2, pp=4, q=4
                )
                for h, src in ((0, f_hi[:].bitcast(f32)), (1, f_lo[:])):
                    srcv = src.rearrange("p (f c) -> p c f", c=IC)
                    for q in range(4):
                        nc.vector.tensor_copy(fview[:, h, q], srcv)

                # ---- psi rows: staging row [1 | x | y | x^2+y^2] built on
                # one partition, then 8 four-partition DMAs tile it into
                # (32, NB) with row 16h+4pp+q = psi_q.  (The old per-row
                # scatter was 28 tiny DMAs/batch whose completion trickled
                # behind the A stream and stalled stage 2 by ~20us.)
                pb_row = s1pool.tile([1, 2 * NB], f32, tag="pbrow")
                nc.scalar.dma_start(
                    pb_row[:], pb[b].rearrange("j k -> (j k)")[None, :]
                )
                grow4 = s1pool.tile([1, 4 * NB], f32, tag="grow")
                nc.vector.memset(grow4[:, 0:NB], 1.0)
                prview = pb_row[:].rearrange("p (j k) -> p k j", k=2)
                nc.vector.tensor_copy(grow4[:, NB : 2 * NB], prview[:, 0, :])
                nc.vector.tensor_copy(grow4[:, 2 * NB : 3 * NB], prview[:, 1, :])
                nc.vector.tensor_mul(grow4[:, 3 * NB : 4 * NB],
                                     grow4[:, NB : 2 * NB],
                                     grow4[:, NB : 2 * NB])
                gtmp = s1pool.tile([1, NB], f32, tag="gtmp")
                nc.vector.tensor_mul(gtmp[:], grow4[:, 2 * NB : 3 * NB],
                                     grow4[:, 2 * NB : 3 * NB])
                nc.vector.tensor_add(grow4[:, 3 * NB : 4 * NB],
                                     grow4[:, 3 * NB : 4 * NB], gtmp[:])
                g_rep = fpool.tile([32, NB], f32, tag="grep")
                gsrc = grow4[0:1, :].rearrange("p (q j) -> p q j", j=NB)
                for r0 in range(0, 32, 4):
                    nc.scalar.dma_start(g_rep[r0 : r0 + 4, :], gsrc)
                feats.append((f_sb, g_rep))

            for b in range(BL):
                f_sb, g_rep = feats[b]
                # ---- stage 1: P32(32, NB) accumulated in 4 one-bank PSUM
                # tiles.  A streams on the sync queue in 2-chunk (2MB) DMAs:
                # in the permuted i-order, adjacent chunks are adjacent DRAM
                # rows, so each partition reads 16KB contiguous.
                p_banks = [
                    psp.tile([32, NJ], f32, tag=f"p{jc}", name=f"p{jc}")
                    for jc in range(JC)
                ]
                Aview = A[b].rearrange("(p s) j -> p s j", s=IC)
                for ic0 in range(0, IC, DMA_CH):
                    a_t = apool.tile([P, DMA_CH * NB], f32r, tag="a")
                    nc.sync.dma_start(
                        a_t[:].rearrange("p (s j) -> p s j", j=NB),
                        Aview[:, ic0 : ic0 + DMA_CH, :],
                    )
                    for s in range(DMA_CH):
                        ic = ic0 + s
                        lhs = f_sb[:, ic * 32 : (ic + 1) * 32]
                        for jc in range(JC):
                            nc.tensor.matmul(
                                p_banks[jc][:],
                                lhs,
                                a_t[:, s * NB + jc * NJ : s * NB + (jc + 1) * NJ],
                                start=(ic == 0),
                                stop=(ic == IC - 1),
                            )

                if DEBUG_STAGE == 1:
                    # stop after stage 1: dump first 25 cols of P32 row 0
                    dbg = spool.tile([1, 25], f32, tag="dbg")
                    nc.vector.tensor_copy(dbg[:], p_banks[0][0:1, 0:25])
                    nc.scalar.dma_start(
                        qo[b].rearrange("a b -> () (a b)"), dbg[:]
                    )
                    continue

                # ---- stage 2: per-bank multiply against g_rep off PSUM,
                # then reduce -> s32
                w32 = spool.tile([32, NB], f32, tag="w32")
                if USE_TTR:
                    s_sb = spool.tile([32, JC], f32, tag="ssb")
                    for jc in range(JC):
                        nc.vector.tensor_tensor_reduce(
                            w32[:, jc * NJ : (jc + 1) * NJ],
                            p_banks[jc][:],
                            g_rep[:, jc * NJ : (jc + 1) * NJ],
                            1.0,
                            0.0,
                            alu.mult,
                            alu.add,
                            s_sb[:, jc : jc + 1],
                        )
                else:
                    s_sb = spool.tile([32, 1], f32, tag="ssb")
                    for jc in range(JC):
                        nc.vector.tensor_mul(
                            w32[:, jc * NJ : (jc + 1) * NJ],
                            p_banks[jc][:],
                            g_rep[:, jc * NJ : (jc + 1) * NJ],
                        )
                    nc.vector.tensor_reduce(
                        s_sb[:], w32[:], mybir.AxisListType.X, alu.add
                    )
                s_tiles.append(s_sb)

            # ---- stage 3 (epilogue, off the PE stream so batch 1's matmuls
            # are not queued behind it): q = tmatq2^T @ s32 (the duplicated
            # tmat rows fold the hi+lo halves over the contraction)
            for b, s_sb in enumerate(s_tiles):
                q_ps = pss.tile([25, s_sb.shape[1]], f32, tag="q")
                nc.tensor.matmul(q_ps[:], tmat_sb[:], s_sb[:], start=True, stop=True)
                q_sb = spool.tile([25, 1], f32, tag="qsb")
                if USE_TTR:
                    nc.vector.tensor_reduce(
                        q_sb[:], q_ps[:], mybir.AxisListType.X, alu.add
                    )
                else:
                    nc.vector.tensor_copy(q_sb[:], q_ps[:])
                nc.gpsimd.dma_start(qo[b].rearrange("a b -> (a b)"), q_sb[:, 0])

    nc.compile()
    _BUILT = nc
    return nc


def kernel(associations: np.ndarray, pt_in_a: np.ndarray, pt_in_b: np.ndarray
           ) -> np.ndarray:
    global LAST_RESULTS
    from concourse.bass_utils import run_bass_kernel_spmd

    nc = _build()
    tq = _tmatq()
    tmatq = np.concatenate([tq, tq], axis=0)  # (32, 25): folds hi+lo halves
    associations = np.ascontiguousarray(associations, dtype=np.float32)
    pt_in_a = np.ascontiguousarray(pt_in_a, dtype=np.float32)
    pt_in_b = np.ascontiguousarray(pt_in_b, dtype=np.float32)

    in_maps = []
    for c in range(N_CORES):
        sl = slice(c * BL, (c + 1) * BL)
        in_maps.append(
            {
                "associations": associations[sl],
                "pt_in_a": pt_in_a[sl],
                "pt_in_b": pt_in_b[sl],
                "tmatq": tmatq,
            }
        )
    res = run_bass_kernel_spmd(nc, in_maps, list(range(N_CORES)))
    LAST_RESULTS = res
    out = np.concatenate([res.results[c]["q_out"] for c in range(N_CORES)], axis=0)
    return out.astype(np.float32, copy=False)
